# revision 25
# baseline (speedup 1.0000x reference)
"""Causal self-attention (B=4, T=2048, C=2048, H=16) on 8 trn2 NeuronCores.

Sharding: tensor-parallel over heads — 2 heads per core. Every core gets the
full (pre-transposed) activation xT, its 2 heads' slice of Wqkv columns and
Wproj rows, computes a full [B*T, C] partial output, and the host sums the 8
partials (the "all-reduce after output projection" done host-side).

Per-core dataflow (all matmuls on PE, fp16 operands, fp32 PSUM accumulate):
  xT tiles --DMA--> QKV proj -> Q^T,K^T [d,t] + V [t,d]
  S = K^T-block.T @ Q^T chunks (PSUM) -> +causal mask (DVE) -> exp (ACT)
  den = ones128.T @ P (PSUM, pre-broadcast across partitions)
  y^T = sum_k V_k^T-block @ P-block (PSUM)
  y normalized by reciprocal_approx_fast(den) (DVE), then
  out_partial = y^T.T @ Wproj-rows -> gpsimd copy -> DMA out (fp16 partials)

Scheduling: the PE instruction queue is kept dependency-free by (a) running
the output projection one (b,qg) unit behind attention, and (b) deferring
every exp-dependent den/PV matmul pair into a FIFO that is drained one entry
per later independent matmul (next S blocks, QKV chunks, proj) — so the
in-order PE queue never parks on a scalar-engine exp, which would idle the
PE and let HAM re-throttle it to 1.2 GHz.
"""
import numpy as np

B, T, C = 4, 2048, 2048
H, HD = 16, 128
N_CORES = 8
HPC = H // N_CORES          # heads per core = 2
SCALE = float(1.0 / np.sqrt(HD))
NEG = -1e9

MM_DT = "fp16"

_CACHE = {}


def _build_nc():
    import concourse.bass as bass
    from concourse import bacc
    import concourse.tile as tile
    import concourse.mybir as mybir
    from concourse.masks import make_identity
    from contextlib import ExitStack

    f32 = mybir.dt.float32
    f16 = mybir.dt.float16
    wdt = f16
    Exp = mybir.ActivationFunctionType.Exp

    nc = bacc.Bacc("TRN2", target_bir_lowering=False, debug=False,
                   enable_asserts=True, num_devices=N_CORES)

    # Inputs (per-core shards prepared on host)
    xT = nc.dram_tensor("xt", [C, B * T], f16, kind="ExternalInput").ap()
    wqkv = nc.dram_tensor("wqkv", [C, 6 * HD], f16, kind="ExternalInput").ap()
    wproj = nc.dram_tensor("wproj", [HPC * HD, C], f16, kind="ExternalInput").ap()
    out = nc.dram_tensor("out", [B * T, C], f16, kind="ExternalOutput").ap()

    # DRAM views: c-chunked weights / activations
    wqkv_v = wqkv.rearrange("(cc p) (jj d) -> p cc jj d", p=128, d=HD)  # [128,16,6,128]
    wproj_v = wproj.rearrange("(jh p) c -> p jh c", p=128)              # [128,2,2048]
    xT_v = xT.rearrange("(cc p) t -> p cc t", p=128)                    # [128,16,B*T]

    NCC = C // 128        # 16 contraction chunks
    NTCH = T // 512       # 4 t-chunks per batch
    LAG = 2               # S-blocks to run ahead of their den/PV consumers

    with tile.TileContext(nc) as tc, ExitStack() as ctx:
        const = ctx.enter_context(tc.tile_pool(name="const", bufs=1))
        wpool = ctx.enter_context(tc.tile_pool(name="w", bufs=1))
        xtp = ctx.enter_context(tc.tile_pool(name="xt", bufs=3))
        qkvp = ctx.enter_context(tc.tile_pool(name="qkv", bufs=2))
        rp = ctx.enter_context(tc.tile_pool(name="r", bufs=2))
        ptp = ctx.enter_context(tc.tile_pool(name="pt", bufs=2))
        ytp = ctx.enter_context(tc.tile_pool(name="yt", bufs=2))
        ob = ctx.enter_context(tc.tile_pool(name="o", bufs=4))
        psS = ctx.enter_context(tc.tile_pool(name="psS", bufs=3, space="PSUM"))
        psQ = ctx.enter_context(tc.tile_pool(name="psQ", bufs=2, space="PSUM"))
        psV = ctx.enter_context(tc.tile_pool(name="psV", bufs=2, space="PSUM"))
        psD = ctx.enter_context(tc.tile_pool(name="psD", bufs=1, space="PSUM"))

        ident_f = const.tile([128, 128], f32)
        make_identity(nc, ident_f)
        ident_h = const.tile([128, 128], f16)
        nc.scalar.copy(ident_h, ident_f)
        # transposed-orientation causal mask: keep (partition=k_rel) <= (free=q_rel)
        triT = const.tile([128, 128], f32)
        nc.gpsimd.memset(triT, 0.0)
        nc.gpsimd.affine_select(
            out=triT, in_=triT, compare_op=mybir.AluOpType.is_ge, fill=NEG,
            base=0, pattern=[[1, 128]], channel_multiplier=-1)
        ones_sq = const.tile([128, 128], f16)
        nc.vector.memset(ones_sq, 1.0)

        w_sb = wpool.tile([128, NCC, 6, HD], wdt)
        wp_sb = wpool.tile([128, 2, C], wdt)

        # ---- deferred-emission FIFO ----------------------------------
        pend = []
        fins_done = {}

        def drain(n):
            for _ in range(min(n, len(pend))):
                pend.pop(0)()

        # ---- input prefetch ------------------------------------------
        chunks = [(b, tch) for b in range(B) for tch in range(NTCH)]
        xt_fifo = []

        def issue_xt(ci, split=False, eng=None):
            b, tch = chunks[ci]
            t0 = b * T + tch * 512
            xt_t = xtp.tile([128, NCC, 512], wdt, tag="xt")
            if split:
                # startup: per-cc pieces on the sync queue, weight pieces
                # interleaved on the scalar queue, so the first QKV matmul
                # starts as soon as the first ~0.3MB lands.
                for cc in range(NCC):
                    nc.sync.dma_start(xt_t[:, cc, :], xT_v[:, cc, t0:t0 + 512])
                    nc.scalar.dma_start(w_sb[:, cc], wqkv_v[:, cc])
            else:
                (eng or nc.sync).dma_start(xt_t, xT_v[:, :, t0:t0 + 512])
            xt_fifo.append(xt_t)

        def emit_qkv_chunk(b, tch, qkv_tiles, startup=False):
            qt, kt, vt, v = qkv_tiles
            xt_t = xt_fifo.pop(0)
            if startup:
                # cc-outer order with 6 parallel PSUM accumulators so the PE
                # consumes each DMA'd cc piece as it lands (borrow banks from
                # the attention pools, which are idle during the prologue).
                accs = [psQ.tile([128, 512], f32, tag="psQ", name="acc0"),
                        psQ.tile([128, 512], f32, tag="psQ", name="acc1"),
                        psS.tile([128, 512], f32, tag="psS", name="acc2"),
                        psS.tile([128, 512], f32, tag="psS", name="acc3"),
                        psV.tile([128, 512], f32, tag="psV", name="acc4"),
                        psV.tile([128, 512], f32, tag="psV", name="acc5")]
                for cc in range(NCC):
                    for jj in range(6):
                        nc.tensor.matmul(accs[jj], w_sb[:, cc, jj, :],
                                         xt_t[:, cc, :],
                                         start=(cc == 0), stop=(cc == NCC - 1))
                for jj in range(6):
                    dst = (qt, qt, kt, kt, vt, vt)[jj]
                    nc.scalar.copy(dst[:, jj % 2, tch * 512:(tch + 1) * 512],
                                   accs[jj])
            else:
                for jj in range(6):  # q_h0, q_h1, k_h0, k_h1, v_h0, v_h1
                    qk_ps = psQ.tile([128, 512], f32, tag="psQ")
                    for cc in range(NCC):
                        nc.tensor.matmul(qk_ps, w_sb[:, cc, jj, :],
                                         xt_t[:, cc, :],
                                         start=(cc == 0), stop=(cc == NCC - 1))
                    dst = (qt, qt, kt, kt, vt, vt)[jj]
                    nc.scalar.copy(dst[:, jj % 2, tch * 512:(tch + 1) * 512],
                                   qk_ps)
                    drain(2)
            # transpose this chunk's V^T slice -> V [t, d]
            for hh in range(HPC):
                for tb in range(4):
                    tg = tch * 4 + tb
                    vp = psQ.tile([128, 128], f16, tag="psQ")
                    nc.tensor.transpose(
                        vp, vt[:, hh, tg * 128:(tg + 1) * 128], ident_h)
                    nc.vector.tensor_copy(v[:, tg, hh * HD:(hh + 1) * HD], vp)
                    drain(1)

        def emit_attn_unit(b, qg, h, qkv_tiles, yt):
            qt, kt, vt, v = qkv_tiles
            pt_sb = ptp.tile([128, T // 128, 512], f16, tag="pt")
            den_ps = psD.tile([128, 512], f32, tag="psD")
            yt_ps = psV.tile([128, 512], f32, tag="psV")
            nkb = 4 * qg + 4

            def emit_dv(kb):
                kk = kb - 4 * qg
                qs = max(0, kk) * 128
                nc.tensor.matmul(
                    den_ps[:, qs:512], ones_sq, pt_sb[:, kb, qs:512],
                    start=(kb == 0), stop=(kb == nkb - 1))
                nc.tensor.matmul(
                    yt_ps[:, qs:512], v[:, kb, h * HD:(h + 1) * HD],
                    pt_sb[:, kb, qs:512],
                    start=(kb == 0), stop=(kb == nkb - 1))

            def finalize():
                r_sb = rp.tile([128, 512], f32, tag="rsb")
                nc.vector.reciprocal_approx_fast(r_sb, den_ps)
                nc.vector.tensor_mul(yt[:, h, :], yt_ps, r_sb)
                fins_done[(b, qg)] = fins_done.get((b, qg), 0) + 1

            for kb in range(nkb):
                kk = kb - 4 * qg
                qs = max(0, kk) * 128
                st = psS.tile([128, 512], f32, tag="psS")
                nc.tensor.matmul(
                    st[:, qs:512], kt[:, h, kb * 128:(kb + 1) * 128],
                    qt[:, h, qg * 512 + qs:(qg + 1) * 512],
                    start=True, stop=True)
                if kk >= 0:
                    nc.vector.tensor_add(
                        st[:, qs:qs + 128], st[:, qs:qs + 128], triT)
                nc.scalar.activation(
                    pt_sb[:, kb, qs:512], st[:, qs:512], Exp, scale=SCALE)
                pend.append(lambda kb=kb: emit_dv(kb))
                while len(pend) > LAG:
                    drain(1)
            pend.append(finalize)

        def emit_proj(b, qg, yt, final=False):
            # both heads' normalize (reciprocal+mul) must already be emitted
            assert fins_done.get((b, qg), 0) == HPC, (b, qg, fins_done)
            for tt in range(4):
                o_big = ob.tile([128, C], f16, tag="ob")
                r0 = b * T + qg * 512 + tt * 128
                for co in range(4):
                    o_ps = psQ.tile([128, 512], f32, tag="psQ")
                    for jh in range(HPC):
                        nc.tensor.matmul(
                            o_ps, yt[:, jh, tt * 128:(tt + 1) * 128],
                            wp_sb[:, jh, co * 512:(co + 1) * 512],
                            start=(jh == 0), stop=(jh == HPC - 1))
                    # alternate copy engines: DVE copy (~680ns) alone is
                    # slower than the 2-matmul cadence (~430ns) and stalls
                    # the PE on PSUM-bank rotation; scalar is idle here.
                    if co % 2 == 0:
                        nc.vector.tensor_copy(
                            o_big[:, co * 512:(co + 1) * 512], o_ps)
                    else:
                        nc.scalar.copy(
                            o_big[:, co * 512:(co + 1) * 512], o_ps)
                    drain(1)
                    if final:  # fine-grained DMA so the drain tail is short
                        nc.sync.dma_start(
                            out[r0:r0 + 128, co * 512:(co + 1) * 512],
                            o_big[:, co * 512:(co + 1) * 512])
                if not final:
                    nc.sync.dma_start(out[r0:r0 + 128, :], o_big)

        def alloc_qkv_tiles():
            qt = qkvp.tile([128, HPC, T], wdt, tag="qt")
            kt = qkvp.tile([128, HPC, T], wdt, tag="kt")
            vt = qkvp.tile([128, HPC, T], f16, tag="vt")
            v = qkvp.tile([128, T // 128, HPC * HD], f16, tag="v")
            return (qt, kt, vt, v)

        # ---- main schedule -------------------------------------------
        issue_xt(0, split=True)
        issue_xt(1, eng=nc.scalar)
        nc.scalar.dma_start(wp_sb, wproj_v)

        ts = {0: alloc_qkv_tiles()}
        for tch in range(NTCH):
            if tch + 2 < len(chunks):
                issue_xt(tch + 2)
            emit_qkv_chunk(0, tch, ts[0], startup=(tch == 0))

        # Chunks 4..15 spread over units 0..14, skipping the (b,3) units:
        # this leaves QKV filler matmuls in the batch-3 units, whose
        # attention stream otherwise develops small PE gaps that let HAM
        # re-throttle the array for the last ~25us of the kernel.
        sched = []
        ci = 4
        for u in range(16):
            if u % 4 == 3 or ci > 15:
                sched.append(None)
            else:
                sched.append(ci)
                ci += 1
        nissued = 6  # xt chunks whose DMA is already in flight
        pending = None
        for b in range(B):
            for qg in range(4):
                c = sched[b * 4 + qg]
                if c is not None:
                    while nissued <= min(c + 2, len(chunks) - 1):
                        issue_xt(nissued)
                        nissued += 1
                    nb, ntch = chunks[c]
                    if nb not in ts:
                        ts[nb] = alloc_qkv_tiles()
                    emit_qkv_chunk(nb, ntch, ts[nb])
                yt = ytp.tile([128, HPC, 512], wdt, tag="yt")
                for h in range(HPC):
                    emit_attn_unit(b, qg, h, ts[b], yt)
                if pending is not None:
                    emit_proj(*pending)
                pending = (b, qg, yt)
        # Final unit: project h0's contribution while h1's softmax tail
        # (last exps + reciprocal + normalize) completes, then add h1's
        # contribution and store — keeps the PE fed through the tail.
        fb, fqg, fyt = pending
        assert fins_done.get((fb, fqg), 0) >= 1, fins_done
        obigs = []
        for tt in range(4):
            o_big = ob.tile([128, C], f16, tag="ob", name=f"obf{tt}")
            obigs.append(o_big)
            for co in range(4):
                o_ps = psQ.tile([128, 512], f32, tag="psQ")
                nc.tensor.matmul(
                    o_ps, fyt[:, 0, tt * 128:(tt + 1) * 128],
                    wp_sb[:, 0, co * 512:(co + 1) * 512],
                    start=True, stop=True)
                if co % 2 == 0:
                    nc.vector.tensor_copy(o_big[:, co * 512:(co + 1) * 512], o_ps)
                else:
                    nc.scalar.copy(o_big[:, co * 512:(co + 1) * 512], o_ps)
                drain(1)
        drain(len(pend))
        assert fins_done.get((fb, fqg), 0) == HPC, fins_done
        for tt in range(4):
            r0 = fb * T + fqg * 512 + tt * 128
            for co in range(4):
                o_ps = psQ.tile([128, 512], f32, tag="psQ")
                nc.tensor.matmul(
                    o_ps, fyt[:, 1, tt * 128:(tt + 1) * 128],
                    wp_sb[:, 1, co * 512:(co + 1) * 512],
                    start=True, stop=True)
                nc.vector.tensor_add(
                    obigs[tt][:, co * 512:(co + 1) * 512],
                    obigs[tt][:, co * 512:(co + 1) * 512], o_ps)
                nc.sync.dma_start(
                    out[r0:r0 + 128, co * 512:(co + 1) * 512],
                    obigs[tt][:, co * 512:(co + 1) * 512])

    nc.compile()
    return nc


def _get_nc():
    if "nc" not in _CACHE:
        _CACHE["nc"] = _build_nc()
    return _CACHE["nc"]


def _make_in_maps(x2d, Wqkv, Wproj):
    hdt = np.float16
    xT = np.ascontiguousarray(x2d.T).astype(hdt)  # [C, B*T]
    in_maps = []
    for c in range(N_CORES):
        h0 = c * HPC
        cols = []
        for part in range(3):  # q, k, v blocks of Wqkv columns
            for h in range(HPC):
                j0 = part * C + (h0 + h) * HD
                cols.append(Wqkv[:, j0:j0 + HD])
        wq = np.ascontiguousarray(np.concatenate(cols, axis=1)).astype(hdt)
        wp = np.ascontiguousarray(Wproj[h0 * HD:(h0 + HPC) * HD, :]).astype(hdt)
        in_maps.append({"xt": xT, "wqkv": wq, "wproj": wp})
    return in_maps


def run_shards(in_maps, trace=False):
    from concourse.bass_utils import run_bass_kernel_spmd
    nc = _get_nc()
    last_err = None
    for _attempt in range(3):
        try:
            return run_bass_kernel_spmd(
                nc, in_maps, core_ids=list(range(N_CORES)), trace=trace)
        except Exception as e:  # transient NRT device errors — retry
            last_err = e
            if "UNAVAILABLE" not in str(e) and "UNRECOVERABLE" not in str(e):
                raise
    raise last_err


def kernel(x, Wqkv, Wproj):
    x = np.asarray(x, dtype=np.float32)
    Wqkv = np.asarray(Wqkv, dtype=np.float32)
    Wproj = np.asarray(Wproj, dtype=np.float32)
    x2d = np.ascontiguousarray(x.reshape(B * T, C))

    in_maps = _make_in_maps(x2d, Wqkv, Wproj)
    res = run_shards(in_maps)

    acc = res.results[0]["out"].astype(np.float64)
    for c in range(1, N_CORES):
        acc += res.results[c]["out"]
    return acc.reshape(B, T, C).astype(np.float32)


# revision 28
# speedup vs baseline: 1.0124x; 1.0124x over previous
"""Causal self-attention (B=4, T=2048, C=2048, H=16) on 8 trn2 NeuronCores.

Sharding: tensor-parallel over heads — 2 heads per core. Every core gets the
full (pre-transposed) activation xT, its 2 heads' slice of Wqkv columns and
Wproj rows, computes a full [B*T, C] partial output, and the host sums the 8
partials (the "all-reduce after output projection" done host-side).

Per-core dataflow (all matmuls on PE, fp16 operands, fp32 PSUM accumulate):
  xT tiles --DMA--> QKV proj -> Q^T,K^T [d,t] + V [t,d]
  S = K^T-block.T @ Q^T chunks (PSUM) -> +causal mask (DVE) -> exp (ACT)
  den = ones128.T @ P (PSUM, pre-broadcast across partitions)
  y^T = sum_k V_k^T-block @ P-block (PSUM)
  y normalized by reciprocal_approx_fast(den) (DVE), then
  out_partial = y^T.T @ Wproj-rows -> gpsimd copy -> DMA out (fp16 partials)

Scheduling: the PE instruction queue is kept dependency-free by (a) running
the output projection one (b,qg) unit behind attention, and (b) deferring
every exp-dependent den/PV matmul pair into a FIFO that is drained one entry
per later independent matmul (next S blocks, QKV chunks, proj) — so the
in-order PE queue never parks on a scalar-engine exp, which would idle the
PE and let HAM re-throttle it to 1.2 GHz.
"""
import numpy as np

B, T, C = 4, 2048, 2048
H, HD = 16, 128
N_CORES = 8
HPC = H // N_CORES          # heads per core = 2
SCALE = float(1.0 / np.sqrt(HD))
NEG = -1e9

MM_DT = "fp16"

_CACHE = {}


def _build_nc():
    import concourse.bass as bass
    from concourse import bacc
    import concourse.tile as tile
    import concourse.mybir as mybir
    from concourse.masks import make_identity
    from contextlib import ExitStack

    f32 = mybir.dt.float32
    f16 = mybir.dt.float16
    wdt = f16
    Exp = mybir.ActivationFunctionType.Exp

    nc = bacc.Bacc("TRN2", target_bir_lowering=False, debug=False,
                   enable_asserts=True, num_devices=N_CORES)

    # Inputs (per-core shards prepared on host)
    xT = nc.dram_tensor("xt", [C, B * T], f16, kind="ExternalInput").ap()
    wqkv = nc.dram_tensor("wqkv", [C, 6 * HD], f16, kind="ExternalInput").ap()
    wproj = nc.dram_tensor("wproj", [HPC * HD, C], f16, kind="ExternalInput").ap()
    out = nc.dram_tensor("out", [B * T, C], f16, kind="ExternalOutput").ap()

    # DRAM views: c-chunked weights / activations
    wqkv_v = wqkv.rearrange("(cc p) (jj d) -> p cc jj d", p=128, d=HD)  # [128,16,6,128]
    wproj_v = wproj.rearrange("(jh p) c -> p jh c", p=128)              # [128,2,2048]
    xT_v = xT.rearrange("(cc p) t -> p cc t", p=128)                    # [128,16,B*T]

    NCC = C // 128        # 16 contraction chunks
    NTCH = T // 512       # 4 t-chunks per batch
    LAG = 2               # S-blocks to run ahead of their den/PV consumers

    with tile.TileContext(nc) as tc, ExitStack() as ctx:
        const = ctx.enter_context(tc.tile_pool(name="const", bufs=1))
        wpool = ctx.enter_context(tc.tile_pool(name="w", bufs=1))
        xtp = ctx.enter_context(tc.tile_pool(name="xt", bufs=3))
        qkvp = ctx.enter_context(tc.tile_pool(name="qkv", bufs=2))
        rp = ctx.enter_context(tc.tile_pool(name="r", bufs=2))
        ptp = ctx.enter_context(tc.tile_pool(name="pt", bufs=2))
        ytp = ctx.enter_context(tc.tile_pool(name="yt", bufs=2))
        ob = ctx.enter_context(tc.tile_pool(name="o", bufs=3))
        psS = ctx.enter_context(tc.tile_pool(name="psS", bufs=3, space="PSUM"))
        psQ = ctx.enter_context(tc.tile_pool(name="psQ", bufs=2, space="PSUM"))
        psV = ctx.enter_context(tc.tile_pool(name="psV", bufs=2, space="PSUM"))
        psD = ctx.enter_context(tc.tile_pool(name="psD", bufs=1, space="PSUM"))

        ident_f = const.tile([128, 128], f32)
        make_identity(nc, ident_f)
        ident_h = const.tile([128, 128], f16)
        nc.scalar.copy(ident_h, ident_f)
        # transposed-orientation causal mask: keep (partition=k_rel) <= (free=q_rel)
        triT = const.tile([128, 128], f32)
        nc.gpsimd.memset(triT, 0.0)
        nc.gpsimd.affine_select(
            out=triT, in_=triT, compare_op=mybir.AluOpType.is_ge, fill=NEG,
            base=0, pattern=[[1, 128]], channel_multiplier=-1)
        ones_sq = const.tile([128, 128], f16)
        nc.vector.memset(ones_sq, 1.0)

        w_sb = wpool.tile([128, NCC, 6, HD], wdt)
        wp_sb = wpool.tile([128, 2, C], wdt)

        # ---- deferred-emission FIFO ----------------------------------
        pend = []
        fins_done = {}

        def drain(n):
            for _ in range(min(n, len(pend))):
                pend.pop(0)()

        # ---- input prefetch ------------------------------------------
        chunks = [(b, tch) for b in range(B) for tch in range(NTCH)]
        xt_fifo = []

        def issue_xt(ci, split=False, eng=None):
            b, tch = chunks[ci]
            t0 = b * T + tch * 512
            xt_t = xtp.tile([128, NCC, 512], wdt, tag="xt")
            if split:
                # startup: per-cc pieces on the sync queue, weight pieces
                # interleaved on the scalar queue, so the first QKV matmul
                # starts as soon as the first ~0.3MB lands.
                for cc in range(NCC):
                    nc.sync.dma_start(xt_t[:, cc, :], xT_v[:, cc, t0:t0 + 512])
                    nc.scalar.dma_start(w_sb[:, cc], wqkv_v[:, cc])
            else:
                (eng or nc.sync).dma_start(xt_t, xT_v[:, :, t0:t0 + 512])
            xt_fifo.append(xt_t)

        def emit_qkv_chunk(b, tch, qkv_tiles, startup=False):
            qt, kt, vt, v = qkv_tiles
            xt_t = xt_fifo.pop(0)
            if startup:
                # cc-outer order with 6 parallel PSUM accumulators so the PE
                # consumes each DMA'd cc piece as it lands (borrow banks from
                # the attention pools, which are idle during the prologue).
                accs = [psQ.tile([128, 512], f32, tag="psQ", name="acc0"),
                        psQ.tile([128, 512], f32, tag="psQ", name="acc1"),
                        psS.tile([128, 512], f32, tag="psS", name="acc2"),
                        psS.tile([128, 512], f32, tag="psS", name="acc3"),
                        psV.tile([128, 512], f32, tag="psV", name="acc4"),
                        psV.tile([128, 512], f32, tag="psV", name="acc5")]
                for cc in range(NCC):
                    for jj in range(6):
                        nc.tensor.matmul(accs[jj], w_sb[:, cc, jj, :],
                                         xt_t[:, cc, :],
                                         start=(cc == 0), stop=(cc == NCC - 1))
                for jj in range(6):
                    dst = (qt, qt, kt, kt, vt, vt)[jj]
                    nc.scalar.copy(dst[:, jj % 2, tch * 512:(tch + 1) * 512],
                                   accs[jj])
            else:
                for jj in range(6):  # q_h0, q_h1, k_h0, k_h1, v_h0, v_h1
                    qk_ps = psQ.tile([128, 512], f32, tag="psQ")
                    for cc in range(NCC):
                        nc.tensor.matmul(qk_ps, w_sb[:, cc, jj, :],
                                         xt_t[:, cc, :],
                                         start=(cc == 0), stop=(cc == NCC - 1))
                    dst = (qt, qt, kt, kt, vt, vt)[jj]
                    nc.scalar.copy(dst[:, jj % 2, tch * 512:(tch + 1) * 512],
                                   qk_ps)
                    drain(2)
            # transpose this chunk's V^T slice -> V [t, d]
            for hh in range(HPC):
                for tb in range(4):
                    tg = tch * 4 + tb
                    vp = psQ.tile([128, 128], f16, tag="psQ")
                    nc.tensor.transpose(
                        vp, vt[:, hh, tg * 128:(tg + 1) * 128], ident_h)
                    nc.vector.tensor_copy(v[:, tg, hh * HD:(hh + 1) * HD], vp)
                    drain(1)

        def emit_attn_unit(b, qg, h, qkv_tiles, yt):
            qt, kt, vt, v = qkv_tiles
            pt_sb = ptp.tile([128, T // 128, 512], f16, tag="pt")
            den_ps = psD.tile([128, 512], f32, tag="psD")
            yt_ps = psV.tile([128, 512], f32, tag="psV")
            nkb = 4 * qg + 4

            def emit_dv(kb):
                kk = kb - 4 * qg
                qs = max(0, kk) * 128
                nc.tensor.matmul(
                    den_ps[:, qs:512], ones_sq, pt_sb[:, kb, qs:512],
                    start=(kb == 0), stop=(kb == nkb - 1))
                nc.tensor.matmul(
                    yt_ps[:, qs:512], v[:, kb, h * HD:(h + 1) * HD],
                    pt_sb[:, kb, qs:512],
                    start=(kb == 0), stop=(kb == nkb - 1))

            def finalize():
                r_sb = rp.tile([128, 512], f32, tag="rsb")
                nc.vector.reciprocal_approx_fast(r_sb, den_ps)
                nc.vector.tensor_mul(yt[:, h, :], yt_ps, r_sb)
                fins_done[(b, qg)] = fins_done.get((b, qg), 0) + 1

            for kb in range(nkb):
                kk = kb - 4 * qg
                qs = max(0, kk) * 128
                st = psS.tile([128, 512], f32, tag="psS")
                nc.tensor.matmul(
                    st[:, qs:512], kt[:, h, kb * 128:(kb + 1) * 128],
                    qt[:, h, qg * 512 + qs:(qg + 1) * 512],
                    start=True, stop=True)
                if kk >= 0:
                    nc.vector.tensor_add(
                        st[:, qs:qs + 128], st[:, qs:qs + 128], triT)
                nc.scalar.activation(
                    pt_sb[:, kb, qs:512], st[:, qs:512], Exp, scale=SCALE)
                pend.append(lambda kb=kb: emit_dv(kb))
                while len(pend) > LAG:
                    drain(1)
            pend.append(finalize)

        def emit_proj(b, qg, yt, final=False):
            # both heads' normalize (reciprocal+mul) must already be emitted
            assert fins_done.get((b, qg), 0) == HPC, (b, qg, fins_done)
            for tt in range(4):
                o_big = ob.tile([128, C], f16, tag="ob")
                r0 = b * T + qg * 512 + tt * 128
                for co in range(4):
                    o_ps = psQ.tile([128, 512], f32, tag="psQ")
                    for jh in range(HPC):
                        nc.tensor.matmul(
                            o_ps, yt[:, jh, tt * 128:(tt + 1) * 128],
                            wp_sb[:, jh, co * 512:(co + 1) * 512],
                            start=(jh == 0), stop=(jh == HPC - 1))
                    # alternate copy engines: DVE copy (~680ns) alone is
                    # slower than the 2-matmul cadence (~430ns) and stalls
                    # the PE on PSUM-bank rotation; scalar is idle here.
                    if co % 2 == 0:
                        nc.vector.tensor_copy(
                            o_big[:, co * 512:(co + 1) * 512], o_ps)
                    else:
                        nc.scalar.copy(
                            o_big[:, co * 512:(co + 1) * 512], o_ps)
                    drain(1)
                    if final:  # fine-grained DMA so the drain tail is short
                        nc.sync.dma_start(
                            out[r0:r0 + 128, co * 512:(co + 1) * 512],
                            o_big[:, co * 512:(co + 1) * 512])
                if not final:
                    nc.sync.dma_start(out[r0:r0 + 128, :], o_big)

        def alloc_qkv_tiles():
            qt = qkvp.tile([128, HPC, T], wdt, tag="qt")
            kt = qkvp.tile([128, HPC, T], wdt, tag="kt")
            vt = qkvp.tile([128, HPC, T], f16, tag="vt")
            v = qkvp.tile([128, T // 128, HPC * HD], f16, tag="v")
            return (qt, kt, vt, v)

        # ---- main schedule -------------------------------------------
        issue_xt(0, split=True)
        nc.scalar.dma_start(wp_sb, wproj_v)
        issue_xt(1)

        tiles = alloc_qkv_tiles()
        for tch in range(NTCH):
            if tch + 2 < len(chunks):
                issue_xt(tch + 2)
            emit_qkv_chunk(0, tch, tiles, startup=(tch == 0))

        pending = None
        for b in range(B):
            nxt = alloc_qkv_tiles() if b + 1 < B else None
            for qg in range(4):
                if nxt is not None:
                    ci = 4 + b * 4 + qg
                    if ci + 2 < len(chunks):
                        issue_xt(ci + 2)
                    emit_qkv_chunk(b + 1, qg, nxt)
                yt = ytp.tile([128, HPC, 512], wdt, tag="yt")
                for h in range(HPC):
                    emit_attn_unit(b, qg, h, tiles, yt)
                if pending is not None:
                    emit_proj(*pending)
                pending = (b, qg, yt)
            tiles = nxt
        drain(len(pend))
        emit_proj(*pending, final=True)

    nc.compile()
    return nc


def _get_nc():
    if "nc" not in _CACHE:
        _CACHE["nc"] = _build_nc()
    return _CACHE["nc"]


def _make_in_maps(x2d, Wqkv, Wproj):
    hdt = np.float16
    xT = np.ascontiguousarray(x2d.T).astype(hdt)  # [C, B*T]
    in_maps = []
    for c in range(N_CORES):
        h0 = c * HPC
        cols = []
        for part in range(3):  # q, k, v blocks of Wqkv columns
            for h in range(HPC):
                j0 = part * C + (h0 + h) * HD
                cols.append(Wqkv[:, j0:j0 + HD])
        wq = np.ascontiguousarray(np.concatenate(cols, axis=1)).astype(hdt)
        wp = np.ascontiguousarray(Wproj[h0 * HD:(h0 + HPC) * HD, :]).astype(hdt)
        in_maps.append({"xt": xT, "wqkv": wq, "wproj": wp})
    return in_maps


def run_shards(in_maps, trace=False):
    from concourse.bass_utils import run_bass_kernel_spmd
    nc = _get_nc()
    last_err = None
    for _attempt in range(3):
        try:
            return run_bass_kernel_spmd(
                nc, in_maps, core_ids=list(range(N_CORES)), trace=trace)
        except Exception as e:  # transient NRT device errors — retry
            last_err = e
            if "UNAVAILABLE" not in str(e) and "UNRECOVERABLE" not in str(e):
                raise
    raise last_err


def kernel(x, Wqkv, Wproj):
    x = np.asarray(x, dtype=np.float32)
    Wqkv = np.asarray(Wqkv, dtype=np.float32)
    Wproj = np.asarray(Wproj, dtype=np.float32)
    x2d = np.ascontiguousarray(x.reshape(B * T, C))

    in_maps = _make_in_maps(x2d, Wqkv, Wproj)
    res = run_shards(in_maps)

    acc = res.results[0]["out"].astype(np.float64)
    for c in range(1, N_CORES):
        acc += res.results[c]["out"]
    return acc.reshape(B, T, C).astype(np.float32)


# revision 29
# speedup vs baseline: 1.0330x; 1.0203x over previous
"""Causal self-attention (B=4, T=2048, C=2048, H=16) on 8 trn2 NeuronCores.

Sharding: tensor-parallel over heads — 2 heads per core. Every core gets the
full (pre-transposed) activation xT, its 2 heads' slice of Wqkv columns and
Wproj rows, computes a full [B*T, C] partial output, and the host sums the 8
partials (the "all-reduce after output projection" done host-side).

Per-core dataflow (all matmuls on PE, fp16 operands, fp32 PSUM accumulate):
  xT tiles --DMA--> QKV proj -> Q^T,K^T [d,t] + V [t,d]
  S = K^T-block.T @ Q^T chunks (PSUM) -> +causal mask (DVE) -> exp (ACT)
  den = ones128.T @ P (PSUM, pre-broadcast across partitions)
  y^T = sum_k V_k^T-block @ P-block (PSUM)
  y normalized by reciprocal_approx_fast(den) (DVE), then
  out_partial = y^T.T @ Wproj-rows -> gpsimd copy -> DMA out (fp16 partials)

Scheduling: the PE instruction queue is kept dependency-free by (a) running
the output projection one (b,qg) unit behind attention, and (b) deferring
every exp-dependent den/PV matmul pair into a FIFO that is drained one entry
per later independent matmul (next S blocks, QKV chunks, proj) — so the
in-order PE queue never parks on a scalar-engine exp, which would idle the
PE and let HAM re-throttle it to 1.2 GHz.
"""
import numpy as np

B, T, C = 4, 2048, 2048
H, HD = 16, 128
N_CORES = 8
HPC = H // N_CORES          # heads per core = 2
SCALE = float(1.0 / np.sqrt(HD))
NEG = -1e9

MM_DT = "fp16"

_CACHE = {}


def _build_nc():
    import concourse.bass as bass
    from concourse import bacc
    import concourse.tile as tile
    import concourse.mybir as mybir
    from concourse.masks import make_identity
    from contextlib import ExitStack

    f32 = mybir.dt.float32
    f16 = mybir.dt.float16
    wdt = f16
    Exp = mybir.ActivationFunctionType.Exp

    nc = bacc.Bacc("TRN2", target_bir_lowering=False, debug=False,
                   enable_asserts=True, num_devices=N_CORES)

    # Inputs (per-core shards prepared on host)
    xT = nc.dram_tensor("xt", [C, B * T], f16, kind="ExternalInput").ap()
    wqkv = nc.dram_tensor("wqkv", [C, 6 * HD], f16, kind="ExternalInput").ap()
    wproj = nc.dram_tensor("wproj", [HPC * HD, C], f16, kind="ExternalInput").ap()
    out = nc.dram_tensor("out", [B * T, C], f16, kind="ExternalOutput").ap()

    # DRAM views: c-chunked weights / activations
    wqkv_v = wqkv.rearrange("(cc p) (jj d) -> p cc jj d", p=128, d=HD)  # [128,16,6,128]
    wproj_v = wproj.rearrange("(jh p) c -> p jh c", p=128)              # [128,2,2048]
    xT_v = xT.rearrange("(cc p) t -> p cc t", p=128)                    # [128,16,B*T]

    NCC = C // 128        # 16 contraction chunks
    NTCH = T // 512       # 4 t-chunks per batch
    LAG = 3               # S-blocks to run ahead of their den/PV consumers

    with tile.TileContext(nc) as tc, ExitStack() as ctx:
        const = ctx.enter_context(tc.tile_pool(name="const", bufs=1))
        wpool = ctx.enter_context(tc.tile_pool(name="w", bufs=1))
        xtp = ctx.enter_context(tc.tile_pool(name="xt", bufs=3))
        qkvp = ctx.enter_context(tc.tile_pool(name="qkv", bufs=2))
        rp = ctx.enter_context(tc.tile_pool(name="r", bufs=2))
        ptp = ctx.enter_context(tc.tile_pool(name="pt", bufs=2))
        ytp = ctx.enter_context(tc.tile_pool(name="yt", bufs=2))
        ob = ctx.enter_context(tc.tile_pool(name="o", bufs=3))
        psS = ctx.enter_context(tc.tile_pool(name="psS", bufs=3, space="PSUM"))
        psQ = ctx.enter_context(tc.tile_pool(name="psQ", bufs=2, space="PSUM"))
        psV = ctx.enter_context(tc.tile_pool(name="psV", bufs=2, space="PSUM"))
        psD = ctx.enter_context(tc.tile_pool(name="psD", bufs=1, space="PSUM"))

        ident_f = const.tile([128, 128], f32)
        make_identity(nc, ident_f)
        ident_h = const.tile([128, 128], f16)
        nc.scalar.copy(ident_h, ident_f)
        # transposed-orientation causal mask: keep (partition=k_rel) <= (free=q_rel)
        triT = const.tile([128, 128], f32)
        nc.gpsimd.memset(triT, 0.0)
        nc.gpsimd.affine_select(
            out=triT, in_=triT, compare_op=mybir.AluOpType.is_ge, fill=NEG,
            base=0, pattern=[[1, 128]], channel_multiplier=-1)
        ones_sq = const.tile([128, 128], f16)
        nc.vector.memset(ones_sq, 1.0)

        w_sb = wpool.tile([128, NCC, 6, HD], wdt)
        wp_sb = wpool.tile([128, 2, C], wdt)

        # ---- deferred-emission FIFO ----------------------------------
        pend = []
        fins_done = {}

        def drain(n):
            for _ in range(min(n, len(pend))):
                pend.pop(0)()

        # ---- input prefetch ------------------------------------------
        chunks = [(b, tch) for b in range(B) for tch in range(NTCH)]
        xt_fifo = []

        def issue_xt(ci, split=False, eng=None):
            b, tch = chunks[ci]
            t0 = b * T + tch * 512
            xt_t = xtp.tile([128, NCC, 512], wdt, tag="xt")
            if split:
                # startup: per-cc pieces on the sync queue, weight pieces
                # interleaved on the scalar queue, so the first QKV matmul
                # starts as soon as the first ~0.3MB lands.
                for cc in range(NCC):
                    nc.sync.dma_start(xt_t[:, cc, :], xT_v[:, cc, t0:t0 + 512])
                    nc.scalar.dma_start(w_sb[:, cc], wqkv_v[:, cc])
            else:
                (eng or nc.sync).dma_start(xt_t, xT_v[:, :, t0:t0 + 512])
            xt_fifo.append(xt_t)

        def emit_qkv_chunk(b, tch, qkv_tiles, startup=False):
            qt, kt, vt, v = qkv_tiles
            xt_t = xt_fifo.pop(0)
            if startup:
                # cc-outer order with 6 parallel PSUM accumulators so the PE
                # consumes each DMA'd cc piece as it lands (borrow banks from
                # the attention pools, which are idle during the prologue).
                accs = [psQ.tile([128, 512], f32, tag="psQ", name="acc0"),
                        psQ.tile([128, 512], f32, tag="psQ", name="acc1"),
                        psS.tile([128, 512], f32, tag="psS", name="acc2"),
                        psS.tile([128, 512], f32, tag="psS", name="acc3"),
                        psV.tile([128, 512], f32, tag="psV", name="acc4"),
                        psV.tile([128, 512], f32, tag="psV", name="acc5")]
                for cc in range(NCC):
                    for jj in range(6):
                        nc.tensor.matmul(accs[jj], w_sb[:, cc, jj, :],
                                         xt_t[:, cc, :],
                                         start=(cc == 0), stop=(cc == NCC - 1))
                for jj in range(6):
                    dst = (qt, qt, kt, kt, vt, vt)[jj]
                    nc.scalar.copy(dst[:, jj % 2, tch * 512:(tch + 1) * 512],
                                   accs[jj])
            else:
                for jj in range(6):  # q_h0, q_h1, k_h0, k_h1, v_h0, v_h1
                    qk_ps = psQ.tile([128, 512], f32, tag="psQ")
                    for cc in range(NCC):
                        nc.tensor.matmul(qk_ps, w_sb[:, cc, jj, :],
                                         xt_t[:, cc, :],
                                         start=(cc == 0), stop=(cc == NCC - 1))
                    dst = (qt, qt, kt, kt, vt, vt)[jj]
                    nc.scalar.copy(dst[:, jj % 2, tch * 512:(tch + 1) * 512],
                                   qk_ps)
                    drain(2)
            # transpose this chunk's V^T slice -> V [t, d]
            for hh in range(HPC):
                for tb in range(4):
                    tg = tch * 4 + tb
                    vp = psQ.tile([128, 128], f16, tag="psQ")
                    nc.tensor.transpose(
                        vp, vt[:, hh, tg * 128:(tg + 1) * 128], ident_h)
                    nc.vector.tensor_copy(v[:, tg, hh * HD:(hh + 1) * HD], vp)
                    drain(1)

        def emit_attn_unit(b, qg, h, qkv_tiles, yt):
            qt, kt, vt, v = qkv_tiles
            pt_sb = ptp.tile([128, T // 128, 512], f16, tag="pt")
            den_ps = psD.tile([128, 512], f32, tag="psD")
            yt_ps = psV.tile([128, 512], f32, tag="psV")
            nkb = 4 * qg + 4

            def emit_dv(kb):
                kk = kb - 4 * qg
                qs = max(0, kk) * 128
                nc.tensor.matmul(
                    den_ps[:, qs:512], ones_sq, pt_sb[:, kb, qs:512],
                    start=(kb == 0), stop=(kb == nkb - 1))
                nc.tensor.matmul(
                    yt_ps[:, qs:512], v[:, kb, h * HD:(h + 1) * HD],
                    pt_sb[:, kb, qs:512],
                    start=(kb == 0), stop=(kb == nkb - 1))

            def finalize():
                r_sb = rp.tile([128, 512], f32, tag="rsb")
                nc.vector.reciprocal_approx_fast(r_sb, den_ps)
                nc.vector.tensor_mul(yt[:, h, :], yt_ps, r_sb)
                fins_done[(b, qg)] = fins_done.get((b, qg), 0) + 1

            for kb in range(nkb):
                kk = kb - 4 * qg
                qs = max(0, kk) * 128
                st = psS.tile([128, 512], f32, tag="psS")
                nc.tensor.matmul(
                    st[:, qs:512], kt[:, h, kb * 128:(kb + 1) * 128],
                    qt[:, h, qg * 512 + qs:(qg + 1) * 512],
                    start=True, stop=True)
                if kk >= 0:
                    nc.vector.tensor_add(
                        st[:, qs:qs + 128], st[:, qs:qs + 128], triT)
                nc.scalar.activation(
                    pt_sb[:, kb, qs:512], st[:, qs:512], Exp, scale=SCALE)
                pend.append(lambda kb=kb: emit_dv(kb))
                while len(pend) > LAG:
                    drain(1)
            pend.append(finalize)

        def emit_proj(b, qg, yt, final=False):
            # both heads' normalize (reciprocal+mul) must already be emitted
            assert fins_done.get((b, qg), 0) == HPC, (b, qg, fins_done)
            for tt in range(4):
                o_big = ob.tile([128, C], f16, tag="ob")
                r0 = b * T + qg * 512 + tt * 128
                for co in range(4):
                    o_ps = psQ.tile([128, 512], f32, tag="psQ")
                    for jh in range(HPC):
                        nc.tensor.matmul(
                            o_ps, yt[:, jh, tt * 128:(tt + 1) * 128],
                            wp_sb[:, jh, co * 512:(co + 1) * 512],
                            start=(jh == 0), stop=(jh == HPC - 1))
                    # alternate copy engines: DVE copy (~680ns) alone is
                    # slower than the 2-matmul cadence (~430ns) and stalls
                    # the PE on PSUM-bank rotation; scalar is idle here.
                    if co % 2 == 0:
                        nc.vector.tensor_copy(
                            o_big[:, co * 512:(co + 1) * 512], o_ps)
                    else:
                        nc.scalar.copy(
                            o_big[:, co * 512:(co + 1) * 512], o_ps)
                    drain(1)
                    if final:  # fine-grained DMA so the drain tail is short
                        nc.sync.dma_start(
                            out[r0:r0 + 128, co * 512:(co + 1) * 512],
                            o_big[:, co * 512:(co + 1) * 512])
                if not final:
                    nc.sync.dma_start(out[r0:r0 + 128, :], o_big)

        def alloc_qkv_tiles():
            qt = qkvp.tile([128, HPC, T], wdt, tag="qt")
            kt = qkvp.tile([128, HPC, T], wdt, tag="kt")
            vt = qkvp.tile([128, HPC, T], f16, tag="vt")
            v = qkvp.tile([128, T // 128, HPC * HD], f16, tag="v")
            return (qt, kt, vt, v)

        # ---- main schedule -------------------------------------------
        issue_xt(0, split=True)
        nc.scalar.dma_start(wp_sb, wproj_v)
        issue_xt(1)

        tiles = alloc_qkv_tiles()
        for tch in range(NTCH):
            if tch + 2 < len(chunks):
                issue_xt(tch + 2)
            emit_qkv_chunk(0, tch, tiles, startup=(tch == 0))

        pending = None
        for b in range(B):
            nxt = alloc_qkv_tiles() if b + 1 < B else None
            for qg in range(4):
                if nxt is not None:
                    ci = 4 + b * 4 + qg
                    if ci + 2 < len(chunks):
                        issue_xt(ci + 2)
                    emit_qkv_chunk(b + 1, qg, nxt)
                yt = ytp.tile([128, HPC, 512], wdt, tag="yt")
                for h in range(HPC):
                    emit_attn_unit(b, qg, h, tiles, yt)
                if pending is not None:
                    emit_proj(*pending)
                pending = (b, qg, yt)
            tiles = nxt
        drain(len(pend))
        emit_proj(*pending, final=True)

    nc.compile()
    return nc


def _get_nc():
    if "nc" not in _CACHE:
        _CACHE["nc"] = _build_nc()
    return _CACHE["nc"]


def _make_in_maps(x2d, Wqkv, Wproj):
    hdt = np.float16
    xT = np.ascontiguousarray(x2d.T).astype(hdt)  # [C, B*T]
    in_maps = []
    for c in range(N_CORES):
        h0 = c * HPC
        cols = []
        for part in range(3):  # q, k, v blocks of Wqkv columns
            for h in range(HPC):
                j0 = part * C + (h0 + h) * HD
                cols.append(Wqkv[:, j0:j0 + HD])
        wq = np.ascontiguousarray(np.concatenate(cols, axis=1)).astype(hdt)
        wp = np.ascontiguousarray(Wproj[h0 * HD:(h0 + HPC) * HD, :]).astype(hdt)
        in_maps.append({"xt": xT, "wqkv": wq, "wproj": wp})
    return in_maps


def run_shards(in_maps, trace=False):
    from concourse.bass_utils import run_bass_kernel_spmd
    nc = _get_nc()
    last_err = None
    for _attempt in range(3):
        try:
            return run_bass_kernel_spmd(
                nc, in_maps, core_ids=list(range(N_CORES)), trace=trace)
        except Exception as e:  # transient NRT device errors — retry
            last_err = e
            if "UNAVAILABLE" not in str(e) and "UNRECOVERABLE" not in str(e):
                raise
    raise last_err


def kernel(x, Wqkv, Wproj):
    x = np.asarray(x, dtype=np.float32)
    Wqkv = np.asarray(Wqkv, dtype=np.float32)
    Wproj = np.asarray(Wproj, dtype=np.float32)
    x2d = np.ascontiguousarray(x.reshape(B * T, C))

    in_maps = _make_in_maps(x2d, Wqkv, Wproj)
    res = run_shards(in_maps)

    acc = res.results[0]["out"].astype(np.float64)
    for c in range(1, N_CORES):
        acc += res.results[c]["out"]
    return acc.reshape(B, T, C).astype(np.float32)
